# revision 2
# baseline (speedup 1.0000x reference)
"""Trainium2 Bass kernel for nn_Net_42176578846907.

Model being implemented (see the problem's reference):
    theta = arctan(x @ W.T + b)                     # (B, 10)
    out   = circuit(theta)                          # (B, 10)
where circuit is a 10-qubit state-vector simulation:
    |0..0> -> H on every qubit -> RX(theta_q) on qubit q -> CNOT ring
    -> <Z_q> per wire.

Exact algebraic simplification used by this kernel:
  * After the Hadamard layer the state is |+>^10 (every amplitude equal).
  * |+> is the +1 eigenstate of X, so RX(t)|+> = e^{-it/2}|+>: the entire
    RX-encoding layer is a GLOBAL PHASE, independent of which amplitude.
  * CNOT|++> = |++>, so the CNOT ring leaves |+>^10 invariant.
  * <Z_q> on |+> is p(0) - p(1) = 1/2 - 1/2 = 0.
So out == 0 for every finite input, exactly.  This even holds bitwise in
float32: after the H layer all 1024 amplitudes are bitwise identical, the
RX update computes c*v + (-i*s)*v for both halves (float add is
commutative, so both halves stay bitwise identical), CNOTs only permute
equal values, and p0 - p1 subtracts two reductions over bitwise-identical
values with identical tree shapes.  The CPU/XLA reference returns exact
0.0 everywhere (verified: abs-max of the reference output is 0.0).

The kernel therefore performs the exact computation -- write zeros --
data-parallel over the batch: each of the 8 cores owns a 4096-row shard
of the (32768, 10) output, memsets an SBUF tile and DMAs it out.
"""

import numpy as np

_NCORES = 8
_BATCH = 32768
_NQ = 10
_BS = _BATCH // _NCORES        # 4096 rows per core
_P = 128                       # SBUF partitions
_FREE = _BS * _NQ // _P        # 320 f32 per partition

_cached = {}


def _build_nc():
    import concourse.bass as bass
    import concourse.mybir as mybir

    nc = bass.Bass()
    out = nc.dram_tensor("out", [_BS, _NQ], mybir.dt.float32,
                         kind="ExternalOutput")

    with (
        nc.Block() as block,
        nc.semaphore("dma_sem") as dma_sem,
        nc.semaphore("z_sem") as z_sem,
        nc.sbuf_tensor("zbuf", [_P, _FREE], mybir.dt.float32) as zbuf,
    ):
        @block.vector
        def _(vector):
            vector.memset(
                bass.AP(zbuf, 0, [[_FREE, _P], [1, _FREE]]), 0
            ).then_inc(z_sem, 1)

        @block.scalar
        def _(scalar):
            scalar.wait_ge(z_sem, 1)
            # (4096, 10) DRAM viewed as 128 partitions x 320 contiguous f32
            scalar.dma_start(
                bass.AP(out, 0, [[_FREE, _P], [1, _FREE]]),
                bass.AP(zbuf, 0, [[_FREE, _P], [1, _FREE]]),
            ).then_inc(dma_sem, 16)
            scalar.wait_ge(dma_sem, 16)

    return nc


def kernel(x: np.ndarray, W: np.ndarray, b: np.ndarray) -> np.ndarray:
    from concourse.bass_utils import run_bass_kernel_spmd

    assert x.shape == (_BATCH, 128) and W.shape == (_NQ, 128)

    if "nc" not in _cached:
        _cached["nc"] = _build_nc()
    nc = _cached["nc"]

    core_ids = list(range(_NCORES))
    in_maps = [{} for _ in core_ids]
    res = run_bass_kernel_spmd(nc, in_maps, core_ids)
    out = np.concatenate([r["out"] for r in res.results], axis=0)
    return out.astype(np.float32, copy=False)
